# revision 3
# baseline (speedup 1.0000x reference)
"""Trainium2 Bass kernel for 16-head causal MHA — v5.

Sharding: core c -> batch c//2, head-group c%2 (8 heads = 4 pairs).
Host sums the two partial [D, S] outputs per batch.

v4 -> v5 redesign (trace-driven):
  - V is projected DIRECTLY into [keys, V|ones] layout (lhsT = x tile,
    rhs = Wv) — eliminates all PE transposes + vfill copies.  V bias is
    folded host-side into bo' = bo + bv_flat @ Wo.T (exact: softmax
    rows sum to 1).
  - j0-block K/Q for cross attention come from the fp8 path; the fp16
    sidecar (accuracy for short softmax rows) only feeds j0-diagonal
    attention via separate kt6/q6/v6 tiles.  The kernel therefore
    starts on the fp8 path ~2us in (v4 idled ~20us cold).
  - need-driven filler schedule: units are emitted exactly before
    first use (per t-tile granularity for V), opportunistic pops keep
    the PE dense between attention tiles.
  - ones regions memset on DVE/gpsimd (v4 burned ~19us of gpsimd
    memset on the critical path).
  - causal mask via gpsimd affine_select on the exp output (frees the
    PE mask matmuls + cmask LDWEIGHTS; bit-identical rel err).
  - fp16 out-projection (fp8 DR was tested: rel err 4e-2, fails gate).
"""

import sys

for _p in ("/opt/trn_rl_repo", "/root/.axon_site/_ro/trn_rl_repo"):
    if _p not in sys.path:
        sys.path.insert(0, _p)

import os

import numpy as np
import ml_dtypes

import concourse.bacc as bacc
import concourse.mybir as mybir
from concourse import bass_utils
from concourse.masks import make_identity, make_causal_mask
from concourse.tile import TileContext

OUTPROJ_FP8 = int(os.environ.get("K6_OUTPROJ_FP8", "0"))
MASK_GPSIMD = int(os.environ.get("K6_MASK_GPSIMD", "1"))
J0_FP8 = int(os.environ.get("K6_J0_FP8", "0"))

P = 128
S = 2048
D = 1024
H = 16
DK = 64
B = 4
NCORES = 8
HPC = 8
NPAIR = 4
NPG = 2          # pair-groups (2 pairs each)
SB = 512
NSB = S // SB    # 4 j-blocks
TT = S // P      # 16 t-tiles
DT = D // P      # 8 d-tiles
NDP = DT // 2    # 4 DoubleRow d-steps
MASKV = -30000.0

F32 = mybir.dt.float32
F16 = mybir.dt.float16
F8 = mybir.dt.float8e4
AF = mybir.ActivationFunctionType
MUL = mybir.AluOpType.mult
DIV = mybir.AluOpType.divide
DR = mybir.MatmulPerfMode.DoubleRow

NPF8 = ml_dtypes.float8_e4m3


def build_nc(debug=False):
    nc = bacc.Bacc()
    # fp8 fast path
    x8_in = nc.dram_tensor("x8", [P, NSB * NDP * 2 * SB], F8,
                           kind="ExternalInput")
    wq_in = nc.dram_tensor("wq8", [P, NPAIR * 1024], F8, kind="ExternalInput")
    wk_in = nc.dram_tensor("wk8", [P, NPAIR * 1024], F8, kind="ExternalInput")
    wv_in = nc.dram_tensor("wv8r", [P, NPG * NDP * 2 * 256], F8,
                           kind="ExternalInput")
    bq_d = nc.dram_tensor("bq_n", [P, NPAIR], F32, kind="ExternalInput")
    bk_d = nc.dram_tensor("bk_n", [P, NPAIR], F32, kind="ExternalInput")
    # fp16 sidecar (j0-diagonal attention only)
    if not J0_FP8:
        xj0_in = nc.dram_tensor("xj0", [D, SB], F16, kind="ExternalInput")
        wq6_in = nc.dram_tensor("wq6r", [P, NPAIR * DT * P], F16,
                                kind="ExternalInput")
        wk6_in = nc.dram_tensor("wk6r", [P, NPAIR * DT * P], F16,
                                kind="ExternalInput")
        wv6_in = nc.dram_tensor("wv6r", [P, NPG * DT * 256], F16,
                                kind="ExternalInput")
    if OUTPROJ_FP8:
        wo_in = nc.dram_tensor("wo8", [P, NPG * 2 * D], F8,
                               kind="ExternalInput")
    else:
        wo_in = nc.dram_tensor("wo_t", [HPC * DK, D], F16,
                               kind="ExternalInput")
    out = nc.dram_tensor("out_part", [D, S], F16, kind="ExternalOutput")

    OTDT = F8 if OUTPROJ_FP8 else F16

    with TileContext(nc) as tc:
        from contextlib import ExitStack

        with ExitStack() as ctx:
            pool = lambda *a, **k: ctx.enter_context(tc.tile_pool(*a, **k))
            x_pool = pool(name="x", bufs=1)
            xj0_pool = pool(name="xj0", bufs=DT)
            wgt_pool = pool(name="wgt", bufs=2 * NPAIR)
            wv8_pool = pool(name="wv8", bufs=NPG)
            wgt6_pool = pool(name="wgt6", bufs=2 * NPAIR)
            wv6_pool = pool(name="wv6", bufs=NPG)
            wo_pool = pool(name="wo", bufs=1 if OUTPROJ_FP8 else NPAIR)
            qt_pool = pool(name="qt", bufs=NPAIR)
            kt_pool = pool(name="kt", bufs=NPAIR)
            q6_pool = pool(name="q6", bufs=2 * NPAIR)
            v8_pool = pool(name="v8", bufs=NPAIR)
            v6_pool = pool(name="v6", bufs=NPAIR)
            ot_pool = pool(name="ot", bufs=NPG)
            wt_pool = pool(name="wt", bufs=6)
            wt6_pool = pool(name="wt6", bufs=2)
            den_pool = pool(name="den", bufs=3)
            ost_pool = pool(name="ost", bufs=3)
            const_pool = pool(name="const", bufs=1)
            ps_big = pool(name="ps_big", bufs=3, space="PSUM")  # sc + pa
            ps_ms = pool(name="ps_ms", bufs=2, space="PSUM")

            # ---- constants / biases (tiny DMAs first) ----
            if not MASK_GPSIMD:
                ident = const_pool.tile([P, P], F16)
                make_identity(nc, ident[:])
                cmask = const_pool.tile([P, P], F16)
                make_causal_mask(nc, cmask[:], mask_val=MASKV)
            biases = {}
            for nm, src in (("q", bq_d), ("k", bk_d)):
                t = const_pool.tile([P, NPAIR], F32, name=f"b{nm}")
                nc.sync.dma_start(t[:], src[:])
                biases[nm] = t

            # ---- persistent tiles ----
            qts = [qt_pool.tile([P, S], F16, tag="qt", name=f"qt{p}")
                   for p in range(NPAIR)]
            kts = [kt_pool.tile([P, S], F16, tag="kt", name=f"kt{p}")
                   for p in range(NPAIR)]
            if J0_FP8:
                q6s = k6s = None
            else:
                q6s = [q6_pool.tile([P, SB], F16, tag="q6", name=f"q6_{p}")
                       for p in range(NPAIR)]
                k6s = [q6_pool.tile([P, SB], F16, tag="q6", name=f"k6_{p}")
                       for p in range(NPAIR)]
            # v8[p]: [keys 128, t-tile, (V_h0|ones|V_h1|ones)]
            v8 = [v8_pool.tile([P, TT, 256], F8, tag="v8", name=f"v8_{p}")
                  for p in range(NPAIR)]
            v6 = (None if J0_FP8 else
                  [v6_pool.tile([P, 4, 256], F16, tag="v6", name=f"v6_{p}")
                   for p in range(NPAIR)])
            # ots per pair-group: [hd-in-pair 128, pair-in-group 2, S]
            otg = [ot_pool.tile([P, 2, S], OTDT, tag="ot", name=f"otg{g}")
                   for g in range(NPG)]
            # ones columns (V bias handled host-side)
            for p in range(NPAIR):
                eng = nc.vector if p < 2 else nc.gpsimd
                eng.memset(v8[p][:, :, 64:128], 1.0)
                eng.memset(v8[p][:, :, 192:256], 1.0)
                if not J0_FP8:
                    eng.memset(v6[p][:, :, 64:128], 1.0)
                    eng.memset(v6[p][:, :, 192:256], 1.0)

            # ---- DMAs: startup-critical first, sidecar/wo deferred ----
            w8 = {"q": [], "k": []}
            # j-block-major: DMA per block is contiguous (4KB lines)
            xt = x_pool.tile([P, NSB, NDP, 2, SB], F8, name="x8t")

            def load_x8_block(jb):
                nc.sync.dma_start(
                    xt[:, jb],
                    x8_in[:, jb * 4096 : (jb + 1) * 4096].rearrange(
                        "p (a b c) -> p a b c", a=NDP, b=2))

            def load_w8(nm, srcw, p):
                t = wgt_pool.tile([P, NDP, 2, P], F8, tag="wgt",
                                  name=f"w8{nm}{p}")
                nc.sync.dma_start(t[:].rearrange("p a b c -> p (a b c)"),
                                  srcw[:, p * 1024 : (p + 1) * 1024])
                w8[nm].append(t)

            wv8r = []

            def load_wv8(g):
                t = wv8_pool.tile([P, NDP, 2, 256], F8, tag="wv8",
                                  name=f"wv8r{g}")
                nc.sync.dma_start(t[:].rearrange("p a b c -> p (a b c)"),
                                  wv_in[:, g * 2048 : (g + 1) * 2048])
                wv8r.append(t)

            load_x8_block(0)
            load_w8("k", wk_in, 0)
            load_x8_block(3)
            load_w8("q", wq_in, 0)
            load_wv8(0)
            load_x8_block(1)
            load_x8_block(2)
            for p in range(1, NPAIR):
                load_w8("k", wk_in, p)
                load_w8("q", wq_in, p)
            load_wv8(1)

            # deferred DMAs (sidecar + wo), issued as a schedule unit
            xj0 = []
            w6 = {"q": [], "k": []}
            wv6r = []
            wo_tiles = []
            wo8t_box = []

            def dma_sidecar():
                for d in range(DT):
                    t = xj0_pool.tile([P, SB], F16, tag="xj0",
                                      name=f"xj0_{d}")
                    nc.sync.dma_start(t[:], xj0_in[d * P : (d + 1) * P, :])
                    xj0.append(t)
                for nm, srcw in (("k", wk6_in), ("q", wq6_in)):
                    for p in range(NPAIR):
                        t = wgt6_pool.tile([P, DT * P], F16, tag="wgt6",
                                           name=f"w6{nm}{p}")
                        nc.sync.dma_start(
                            t[:], srcw[:, p * DT * P : (p + 1) * DT * P])
                        w6[nm].append(t)
                for g in range(NPG):
                    t = wv6_pool.tile([P, DT, 256], F16, tag="wv6",
                                      name=f"wv6r{g}")
                    nc.sync.dma_start(t[:].rearrange("p a b -> p (a b)"),
                                      wv6_in[:, g * 2048 : (g + 1) * 2048])
                    wv6r.append(t)

            def dma_wo():
                if OUTPROJ_FP8:
                    t = wo_pool.tile([P, NPG, 2, D], F8, tag="wo",
                                     name="wo8t")
                    nc.sync.dma_start(t[:].rearrange("p a b c -> p (a b c)"),
                                      wo_in[:])
                    wo8t_box.append(t)
                else:
                    for pr in range(NPAIR):
                        t = wo_pool.tile([P, D], F16, tag="wo",
                                         name=f"wo{pr}")
                        nc.sync.dma_start(t[:],
                                          wo_in[pr * P : (pr + 1) * P, :])
                        wo_tiles.append(t)

            # ---------- units ----------
            def proj8_unit(nm, p, jb):
                """fp8 DR projection of Q/K block jb -> kt/qt [*, jb*SB:]."""
                ps = ps_ms.tile([P, SB], F32, tag="ms", name=f"ps8{nm}")
                for dp in range(NDP):
                    nc.tensor.matmul(
                        ps[:],
                        w8[nm][p][:, dp, :, :],
                        xt[:, jb, dp, :, :],
                        start=(dp == 0),
                        stop=(dp == NDP - 1),
                        perf_mode=DR,
                    )
                dest = qts[p] if nm == "q" else kts[p]
                nc.vector.tensor_scalar_add(
                    dest[:, jb * SB : (jb + 1) * SB], ps[:],
                    biases[nm][:, p : p + 1])

            def v8_unit(g, t):
                """fp8 DR V projection of t-tile t for pair-group g."""
                ps = ps_ms.tile([P, 256], F32, tag="ms", name="psv")
                for dp in range(NDP):
                    nc.tensor.matmul(
                        ps[:],
                        xt[:, t // 4, dp, :, (t % 4) * P : (t % 4 + 1) * P],
                        wv8r[g][:, dp, :, :],
                        start=(dp == 0),
                        stop=(dp == NDP - 1),
                        perf_mode=DR,
                    )
                for u in range(2):
                    dst = v8[2 * g + u][:, t, :].rearrange(
                        "p (h c) -> p h c", h=2)[:, :, 0:DK]
                    nc.vector.tensor_copy(
                        dst,
                        ps[:, u * P : (u + 1) * P].rearrange(
                            "p (h c) -> p h c", c=DK))

            def v6_unit(g, t):
                """fp16 V projection of t-tile t (j0 diag) for group g."""
                ps = ps_ms.tile([P, 256], F32, tag="ms", name="psv6")
                for d in range(DT):
                    nc.tensor.matmul(
                        ps[:],
                        xj0[d][:, t * P : (t + 1) * P],
                        wv6r[g][:, d, :],
                        start=(d == 0),
                        stop=(d == DT - 1),
                    )
                for u in range(2):
                    dst = v6[2 * g + u][:, t, :].rearrange(
                        "p (h c) -> p h c", h=2)[:, :, 0:DK]
                    nc.vector.tensor_copy(
                        dst,
                        ps[:, u * P : (u + 1) * P].rearrange(
                            "p (h c) -> p h c", c=DK))

            def proj6_unit(nm, p):
                """fp16 sidecar projection of s-block 0 -> q6/k6."""
                ps = ps_ms.tile([P, SB], F32, tag="ms", name=f"ps6{nm}")
                for d in range(DT):
                    nc.tensor.matmul(
                        ps[:],
                        w6[nm][p][:, d * P : (d + 1) * P],
                        xj0[d][:],
                        start=(d == 0),
                        stop=(d == DT - 1),
                    )
                dest = q6s[p] if nm == "q" else k6s[p]
                nc.vector.tensor_scalar_add(
                    dest[:], ps[:], biases[nm][:, p : p + 1])

            def outproj_unit(j, m):
                ps = ps_ms.tile([P, SB], F32, tag="ms", name="ps_o")
                if OUTPROJ_FP8:
                    wo8t = wo8t_box[0]
                    for g in range(NPG):
                        nc.tensor.matmul(
                            ps[:],
                            wo8t[:, g, :, m * P : (m + 1) * P],
                            otg[g][:, :, j * SB : (j + 1) * SB],
                            start=(g == 0),
                            stop=(g == NPG - 1),
                            perf_mode=DR,
                        )
                else:
                    for pr in range(NPAIR):
                        nc.tensor.matmul(
                            ps[:],
                            wo_tiles[pr][:, m * P : (m + 1) * P],
                            otg[pr // 2][:, pr % 2, j * SB : (j + 1) * SB],
                            start=(pr == 0),
                            stop=(pr == NPAIR - 1),
                        )
                st = ost_pool.tile([P, SB], F16, tag="ost")
                nc.vector.tensor_copy(st[:], ps[:])
                nc.sync.dma_start(
                    out[m * P : (m + 1) * P, j * SB : (j + 1) * SB], st[:])

            # ---------- schedule machinery ----------
            units = {}
            phase_q = {3: [], 2: [], 1: [], 0: [], -1: []}

            def add_unit(ph, key, fn):
                units[key] = fn
                phase_q[ph].append(key)

            def need(key):
                fn = units.pop(key, None)
                if fn is not None:
                    fn()

            def pop_one(cur):
                for jj in (3, 2, 1, 0, -1):
                    if jj < cur:
                        break
                    q = phase_q[jj]
                    while q:
                        key = q.pop(0)
                        fn = units.pop(key, None)
                        if fn is not None:
                            fn()
                            return True
                return False

            for p in range(NPAIR):
                for jb in range(NSB):
                    add_unit(3, ("k8", p, jb),
                             lambda p=p, jb=jb: proj8_unit("k", p, jb))
                add_unit(3, ("q8", p, 3),
                         lambda p=p: proj8_unit("q", p, 3))
                if p % 2 == 1:
                    g = p // 2
                    for t in range(TT):
                        add_unit(3, ("v8", g, t),
                                 lambda g=g, t=t: v8_unit(g, t))
            for p in range(NPAIR):
                add_unit(3, ("q8", p, 2),
                         lambda p=p: proj8_unit("q", p, 2))
            add_unit(3, ("dma_wo",), dma_wo)
            if not J0_FP8:
                add_unit(2, ("dma6",), dma_sidecar)
            for p in range(NPAIR):
                add_unit(2, ("q8", p, 1),
                         lambda p=p: proj8_unit("q", p, 1))
            if J0_FP8:
                for p in range(NPAIR):
                    add_unit(1, ("q8", p, 0),
                             lambda p=p: proj8_unit("q", p, 0))
            else:
                for p in range(NPAIR):
                    add_unit(2, ("k6", p), lambda p=p: proj6_unit("k", p))
                for p in range(NPAIR):
                    add_unit(1, ("q6", p), lambda p=p: proj6_unit("q", p))
                for g in range(NPG):
                    for t in range(4):
                        add_unit(1, ("v6", g, t),
                                 lambda g=g, t=t: v6_unit(g, t))

            # ---------- attention ----------
            def attention_j(p, j):
                f16 = j == 0 and not J0_FP8
                g = p // 2
                nt = 4 * j + 4
                ktp = k6s[p] if f16 else kts[p]
                qtp = q6s[p] if f16 else qts[p]
                vtp = v6[p] if f16 else v8[p]
                pa = ps_big.tile([P, 2, SB], F32, tag="big", name=f"pa{p}")
                wts = {}
                pend = []

                if f16:
                    need(("k6", p))
                    need(("q6", p))
                else:
                    need(("q8", p, j))

                def av_emit(i):
                    diag = i >= 4 * j
                    r = i - 4 * j
                    w = SB - P * r if diag else SB
                    q = i % 2
                    wt_t = wts[i // 2]
                    last = i == nt - 1
                    if diag:
                        if f16:
                            need(("v6", g, i))
                        else:
                            need(("v8", g, i))
                        c0 = P * r
                        for h in range(2):
                            nc.tensor.matmul(
                                pa[:, h, c0:SB],
                                vtp[:, i, 128 * h : 128 * h + 128],
                                wt_t[:, h, q, 0:w],
                                start=(i == 0),
                                stop=last,
                            )
                    elif q == 1:
                        need(("v8", g, i - 1))
                        need(("v8", g, i))
                        for h in range(2):
                            nc.tensor.matmul(
                                pa[:, h, :],
                                v8[p][:, i - 1 : i + 1,
                                      128 * h : 128 * h + 128],
                                wt_t[:, h, :, :],
                                start=(i == 1),
                                stop=False,
                                perf_mode=DR,
                            )

                for i in range(nt):
                    if not f16:
                        need(("k8", p, i // 4))
                    pop_one(j)
                    diag = i >= 4 * j
                    r = i - 4 * j
                    w = SB - P * r if diag else SB
                    qoff = j * SB + (P * r if diag else 0)
                    q = i % 2
                    sc = ps_big.tile([P, 2, SB], F32, tag="big", name="sc")
                    for h in range(2):
                        nc.tensor.matmul(
                            sc[:, h, 0:w],
                            ktp[64 * h : 64 * h + 64, i * P : (i + 1) * P],
                            qtp[64 * h : 64 * h + 64, qoff : qoff + w],
                            start=True, stop=bool(MASK_GPSIMD) or not diag,
                        )
                        if diag and not MASK_GPSIMD:
                            nc.tensor.matmul(
                                sc[:, h, 0:P],
                                cmask[:],
                                ident[:],
                                start=False, stop=True,
                                skip_group_check=True,
                            )
                    if q == 0:
                        if f16:
                            wt_cur = wt6_pool.tile([P, 2, 2, SB], F16,
                                                   tag="wt6", name="wt6c")
                        else:
                            wt_cur = wt_pool.tile([P, 2, 2, SB], F8,
                                                  tag="wt", name="wt8c")
                        wts[i // 2] = wt_cur
                    nc.scalar.activation(
                        wts[i // 2][:, :, q, 0:w], sc[:, :, 0:w], AF.Exp,
                        scale=0.125,
                    )
                    if diag and MASK_GPSIMD:
                        # zero exp(score) where key > query (upper triangle
                        # of the leading 128-col block)
                        nc.gpsimd.affine_select(
                            out=wts[i // 2][:, :, q, 0:P],
                            in_=wts[i // 2][:, :, q, 0:P],
                            compare_op=mybir.AluOpType.is_ge,
                            fill=0.0,
                            base=0,
                            pattern=[[0, 2], [1, P]],
                            channel_multiplier=-1,
                        )
                    pend.append(i)
                    # defer AV by 3 tiles so the PE never waits on exp
                    if len(pend) > 3:
                        av_emit(pend.pop(0))
                for i in pend:
                    av_emit(i)
                den = den_pool.tile([DK, 2, SB], F32, tag="den", name="den")
                nc.vector.tensor_copy(den[:], pa[DK:P, :, :])
                num = den_pool.tile([DK, 2, SB], F32, tag="num", name="num")
                nc.vector.tensor_copy(num[:], pa[0:DK, :, :])
                # pa slot is free from here: sc/pa pool rotation unblocks
                dst = otg[g]
                rcs = den_pool.tile([DK, 2, SB], F32, tag="den", name="rcs")
                nc.vector.reciprocal_approx_fast(rcs[:], den[:])
                for h in range(2):
                    nc.vector.tensor_tensor(
                        dst[h * DK : (h + 1) * DK, p % 2,
                            j * SB : (j + 1) * SB],
                        num[:, h, :],
                        rcs[:, h, :],
                        MUL,
                    )

            # ---------- main loop ----------
            for j in range(NSB - 1, -1, -1):
                for p in range(NPAIR):
                    attention_j(p, j)
                for m in range(DT):
                    add_unit(max(j - 1, -1), ("op", j, m),
                             lambda j=j, m=m: outproj_unit(j, m))
            while pop_one(-1):
                pass

    nc.compile()
    return nc


_NC_CACHE = None


def _get_nc():
    global _NC_CACHE
    if _NC_CACHE is None:
        _NC_CACHE = build_nc()
    return _NC_CACHE


def _core_inputs(x, Wq, bq, Wk, bk, Wv, bv, Wo, c):
    b, g = c // 2, c % 2
    heads = list(range(g * HPC, (g + 1) * HPC))
    out = {}
    xT = np.ascontiguousarray(x[b].T)  # [D, S] f32
    # x8[p, jb, dp, slot, s] = xT[dp*256 + slot*128 + p, jb*512 + s]
    x8 = xT.reshape(NDP, 2, P, NSB, SB).transpose(2, 3, 0, 1, 4)
    out["x8"] = np.ascontiguousarray(
        x8.reshape(P, NSB * NDP * 2 * SB).astype(NPF8))

    def pack_w8(W):
        cols = np.empty((P, NPAIR, NDP, 2, P), dtype=np.float32)
        for p in range(NPAIR):
            hA, hB = heads[2 * p], heads[2 * p + 1]
            Wp = np.concatenate([W[hA], W[hB]], axis=1)  # [1024, 128]
            Wp = Wp.reshape(NDP, 2, P, P)
            cols[:, p] = Wp.transpose(2, 0, 1, 3)
        return np.ascontiguousarray(cols.reshape(P, NPAIR * 1024).astype(NPF8))

    def pack_b(bias):
        cols = np.empty((P, NPAIR), dtype=np.float32)
        for p in range(NPAIR):
            hA, hB = heads[2 * p], heads[2 * p + 1]
            cols[:, p] = np.concatenate([bias[hA], bias[hB]])
        return np.ascontiguousarray(cols)

    out["wq8"] = pack_w8(Wq)
    out["wk8"] = pack_w8(Wk)
    out["bq_n"] = pack_b(bq)
    out["bk_n"] = pack_b(bk)

    # wv8r[p, g, dp, slot, n] = Wv4[g][dp*256 + slot*128 + p, n]
    wv8r = np.empty((P, NPG, NDP, 2, 256), dtype=np.float32)
    wv6r = np.empty((P, NPG, DT, 256), dtype=np.float32)
    for gg in range(NPG):
        hs = heads[4 * gg : 4 * gg + 4]
        Wv4 = np.concatenate([Wv[h] for h in hs], axis=1)  # [1024, 256]
        wv8r[:, gg] = Wv4.reshape(NDP, 2, P, 256).transpose(2, 0, 1, 3)
        if not J0_FP8:
            wv6r[:, gg] = Wv4.reshape(DT, P, 256).transpose(1, 0, 2)
    out["wv8r"] = np.ascontiguousarray(
        wv8r.reshape(P, NPG * NDP * 2 * 256).astype(NPF8))
    if not J0_FP8:
        out["wv6r"] = np.ascontiguousarray(
            wv6r.reshape(P, NPG * DT * 256).astype(np.float16))

    if not J0_FP8:
        out["xj0"] = np.ascontiguousarray(xT[:, 0:SB].astype(np.float16))

    def cat16r(W):
        # [P, pair, d, c]: w6r[p, pr, d, c] = Wpair[d*128 + p, c]
        cols = np.empty((P, NPAIR, DT, P), dtype=np.float32)
        for pr in range(NPAIR):
            Wp = np.concatenate([W[heads[2 * pr]], W[heads[2 * pr + 1]]],
                                axis=1)  # [1024, 128]
            cols[:, pr] = Wp.reshape(DT, P, P).transpose(1, 0, 2)
        return np.ascontiguousarray(
            cols.reshape(P, NPAIR * DT * P).astype(np.float16))
    if not J0_FP8:
        out["wq6r"] = cat16r(Wq)
        out["wk6r"] = cat16r(Wk)

    Wo_core = Wo[:, g * HPC * DK : (g + 1) * HPC * DK].T  # [512, 1024]
    if OUTPROJ_FP8:
        # wo8[p, g, slot, m] = Wo_core[g*256 + slot*128 + p, m]
        wo8 = Wo_core.reshape(NPG, 2, P, D).transpose(2, 0, 1, 3)
        out["wo8"] = np.ascontiguousarray(
            wo8.reshape(P, NPG * 2 * D).astype(NPF8))
    else:
        out["wo_t"] = np.ascontiguousarray(Wo_core.astype(np.float16))
    return out


def kernel(x, Wq, bq, Wk, bk, Wv, bv, Wo, bo, _trace=False, _tmpdir=None):
    x = np.asarray(x, dtype=np.float32)
    nc = _get_nc()
    in_maps = [
        _core_inputs(x, Wq, bq, Wk, bk, Wv, bv, Wo, c) for c in range(NCORES)
    ]
    kw = {}
    if _trace:
        kw = dict(trace=True, tmpdir=_tmpdir)
    res = bass_utils.run_bass_kernel_spmd(
        nc, in_maps, core_ids=list(range(NCORES)), **kw
    )
    # V bias folded here: softmax rows sum to 1 -> +bv passes through AV
    bo_eff = (np.asarray(bo, dtype=np.float32)
              + np.asarray(bv, dtype=np.float32).reshape(-1)
              @ np.asarray(Wo, dtype=np.float32).T)
    out = np.empty((B, S, D), dtype=np.float32)
    for b in range(B):
        part = (res.results[2 * b]["out_part"].astype(np.float32)
                + res.results[2 * b + 1]["out_part"].astype(np.float32))
        out[b] = part.T + bo_eff
    if _trace:
        kernel._last_results = res
    return out


# revision 6
# speedup vs baseline: 1.0130x; 1.0130x over previous
"""Trainium2 Bass kernel for 16-head causal MHA — v5.

Sharding: core c -> batch c//2, head-group c%2 (8 heads = 4 pairs).
Host sums the two partial [D, S] outputs per batch.

v4 -> v5 redesign (trace-driven):
  - V is projected DIRECTLY into [keys, V|ones] layout (lhsT = x tile,
    rhs = Wv) — eliminates all PE transposes + vfill copies.  V bias is
    folded host-side into bo' = bo + bv_flat @ Wo.T (exact: softmax
    rows sum to 1).
  - j0-block K/Q for cross attention come from the fp8 path; the fp16
    sidecar (accuracy for short softmax rows) only feeds j0-diagonal
    attention via separate kt6/q6/v6 tiles.  The kernel therefore
    starts on the fp8 path ~2us in (v4 idled ~20us cold).
  - need-driven filler schedule: units are emitted exactly before
    first use (per t-tile granularity for V), opportunistic pops keep
    the PE dense between attention tiles.
  - ones regions memset on DVE/gpsimd (v4 burned ~19us of gpsimd
    memset on the critical path).
  - causal mask via gpsimd affine_select on the exp output (frees the
    PE mask matmuls + cmask LDWEIGHTS; bit-identical rel err).
  - fp16 out-projection (fp8 DR was tested: rel err 4e-2, fails gate).
"""

import sys

for _p in ("/opt/trn_rl_repo", "/root/.axon_site/_ro/trn_rl_repo"):
    if _p not in sys.path:
        sys.path.insert(0, _p)

import os

import numpy as np
import ml_dtypes

import concourse.bacc as bacc
import concourse.mybir as mybir
from concourse import bass_utils
from concourse.masks import make_identity, make_causal_mask
from concourse.tile import TileContext

OUTPROJ_FP8 = int(os.environ.get("K6_OUTPROJ_FP8", "0"))
MASK_GPSIMD = int(os.environ.get("K6_MASK_GPSIMD", "1"))
J0_FP8 = int(os.environ.get("K6_J0_FP8", "0"))

P = 128
S = 2048
D = 1024
H = 16
DK = 64
B = 4
NCORES = 8
HPC = 8
NPAIR = 4
NPG = 2          # pair-groups (2 pairs each)
SB = 512
NSB = S // SB    # 4 j-blocks
TT = S // P      # 16 t-tiles
DT = D // P      # 8 d-tiles
NDP = DT // 2    # 4 DoubleRow d-steps
MASKV = -30000.0

F32 = mybir.dt.float32
F16 = mybir.dt.float16
F8 = mybir.dt.float8e4
AF = mybir.ActivationFunctionType
MUL = mybir.AluOpType.mult
DIV = mybir.AluOpType.divide
DR = mybir.MatmulPerfMode.DoubleRow

NPF8 = ml_dtypes.float8_e4m3


def build_nc(debug=False):
    nc = bacc.Bacc()
    # fp8 fast path
    x8_in = nc.dram_tensor("x8", [P, NSB * NDP * 2 * SB], F8,
                           kind="ExternalInput")
    wq_in = nc.dram_tensor("wq8", [P, NPAIR * 1024], F8, kind="ExternalInput")
    wk_in = nc.dram_tensor("wk8", [P, NPAIR * 1024], F8, kind="ExternalInput")
    wv_in = nc.dram_tensor("wv8r", [P, NPG * NDP * 2 * 256], F8,
                           kind="ExternalInput")
    bq_d = nc.dram_tensor("bq_n", [P, NPAIR], F32, kind="ExternalInput")
    bk_d = nc.dram_tensor("bk_n", [P, NPAIR], F32, kind="ExternalInput")
    # fp16 sidecar (j0-diagonal attention only)
    if not J0_FP8:
        xj0_in = nc.dram_tensor("xj0", [D, SB], F16, kind="ExternalInput")
        wq6_in = nc.dram_tensor("wq6r", [P, NPAIR * DT * P], F16,
                                kind="ExternalInput")
        wk6_in = nc.dram_tensor("wk6r", [P, NPAIR * DT * P], F16,
                                kind="ExternalInput")
        wv6_in = nc.dram_tensor("wv6r", [P, NPG * DT * 256], F16,
                                kind="ExternalInput")
    if OUTPROJ_FP8:
        wo_in = nc.dram_tensor("wo8", [P, NPG * 2 * D], F8,
                               kind="ExternalInput")
    else:
        wo_in = nc.dram_tensor("wo_t", [HPC * DK, D], F16,
                               kind="ExternalInput")
    out = nc.dram_tensor("out_part", [D, S], F16, kind="ExternalOutput")

    OTDT = F8 if OUTPROJ_FP8 else F16

    with TileContext(nc) as tc:
        from contextlib import ExitStack

        with ExitStack() as ctx:
            pool = lambda *a, **k: ctx.enter_context(tc.tile_pool(*a, **k))
            x_pool = pool(name="x", bufs=1)
            xj0_pool = pool(name="xj0", bufs=DT)
            wgt_pool = pool(name="wgt", bufs=2 * NPAIR)
            wv8_pool = pool(name="wv8", bufs=NPG)
            wgt6_pool = pool(name="wgt6", bufs=2 * NPAIR)
            wv6_pool = pool(name="wv6", bufs=NPG)
            wo_pool = pool(name="wo", bufs=1 if OUTPROJ_FP8 else NPAIR)
            qt_pool = pool(name="qt", bufs=NPAIR)
            kt_pool = pool(name="kt", bufs=NPAIR)
            q6_pool = pool(name="q6", bufs=2 * NPAIR)
            v8_pool = pool(name="v8", bufs=NPAIR)
            v6_pool = pool(name="v6", bufs=NPAIR)
            ot_pool = pool(name="ot", bufs=NPG)
            wt_pool = pool(name="wt", bufs=7)
            wt6_pool = pool(name="wt6", bufs=2)
            den_pool = pool(name="den", bufs=4)
            ost_pool = pool(name="ost", bufs=4)
            const_pool = pool(name="const", bufs=1)
            ps_big = pool(name="ps_big", bufs=3, space="PSUM")  # sc + pa
            ps_ms = pool(name="ps_ms", bufs=2, space="PSUM")

            # ---- constants / biases (tiny DMAs first) ----
            if not MASK_GPSIMD:
                ident = const_pool.tile([P, P], F16)
                make_identity(nc, ident[:])
                cmask = const_pool.tile([P, P], F16)
                make_causal_mask(nc, cmask[:], mask_val=MASKV)
            biases = {}
            for nm, src in (("q", bq_d), ("k", bk_d)):
                t = const_pool.tile([P, NPAIR], F32, name=f"b{nm}")
                nc.sync.dma_start(t[:], src[:])
                biases[nm] = t

            # ---- persistent tiles ----
            qts = [qt_pool.tile([P, S], F16, tag="qt", name=f"qt{p}")
                   for p in range(NPAIR)]
            kts = [kt_pool.tile([P, S], F16, tag="kt", name=f"kt{p}")
                   for p in range(NPAIR)]
            if J0_FP8:
                q6s = k6s = None
            else:
                q6s = [q6_pool.tile([P, SB], F16, tag="q6", name=f"q6_{p}")
                       for p in range(NPAIR)]
                k6s = [q6_pool.tile([P, SB], F16, tag="q6", name=f"k6_{p}")
                       for p in range(NPAIR)]
            # v8[p]: [keys 128, t-tile, (V_h0|ones|V_h1|ones)]
            v8 = [v8_pool.tile([P, TT, 256], F8, tag="v8", name=f"v8_{p}")
                  for p in range(NPAIR)]
            v6 = (None if J0_FP8 else
                  [v6_pool.tile([P, 4, 256], F16, tag="v6", name=f"v6_{p}")
                   for p in range(NPAIR)])
            # ots per pair-group: [hd-in-pair 128, pair-in-group 2, S]
            otg = [ot_pool.tile([P, 2, S], OTDT, tag="ot", name=f"otg{g}")
                   for g in range(NPG)]
            # ones columns (V bias handled host-side)
            for p in range(NPAIR):
                eng = nc.vector if p < 2 else nc.gpsimd
                eng.memset(v8[p][:, :, 64:128], 1.0)
                eng.memset(v8[p][:, :, 192:256], 1.0)
                if not J0_FP8:
                    eng.memset(v6[p][:, :, 64:128], 1.0)
                    eng.memset(v6[p][:, :, 192:256], 1.0)

            # ---- DMAs: startup-critical first, sidecar/wo deferred ----
            w8 = {"q": [], "k": []}
            # j-block-major: DMA per block is contiguous (4KB lines)
            xt = x_pool.tile([P, NSB, NDP, 2, SB], F8, name="x8t")

            def load_x8_block(jb):
                nc.sync.dma_start(
                    xt[:, jb],
                    x8_in[:, jb * 4096 : (jb + 1) * 4096].rearrange(
                        "p (a b c) -> p a b c", a=NDP, b=2))

            def load_w8(nm, srcw, p):
                t = wgt_pool.tile([P, NDP, 2, P], F8, tag="wgt",
                                  name=f"w8{nm}{p}")
                nc.sync.dma_start(t[:].rearrange("p a b c -> p (a b c)"),
                                  srcw[:, p * 1024 : (p + 1) * 1024])
                w8[nm].append(t)

            wv8r = []

            def load_wv8(g):
                t = wv8_pool.tile([P, NDP, 2, 256], F8, tag="wv8",
                                  name=f"wv8r{g}")
                nc.sync.dma_start(t[:].rearrange("p a b c -> p (a b c)"),
                                  wv_in[:, g * 2048 : (g + 1) * 2048])
                wv8r.append(t)

            load_x8_block(0)
            load_w8("k", wk_in, 0)
            load_x8_block(3)
            load_w8("q", wq_in, 0)
            load_wv8(0)
            load_x8_block(1)
            load_x8_block(2)
            for p in range(1, NPAIR):
                load_w8("k", wk_in, p)
                load_w8("q", wq_in, p)
            load_wv8(1)

            # deferred DMAs (sidecar + wo), issued as a schedule unit
            xj0 = []
            w6 = {"q": [], "k": []}
            wv6r = []
            wo_tiles = []
            wo8t_box = []

            def dma_sidecar():
                for d in range(DT):
                    t = xj0_pool.tile([P, SB], F16, tag="xj0",
                                      name=f"xj0_{d}")
                    nc.sync.dma_start(t[:], xj0_in[d * P : (d + 1) * P, :])
                    xj0.append(t)
                for nm, srcw in (("k", wk6_in), ("q", wq6_in)):
                    for p in range(NPAIR):
                        t = wgt6_pool.tile([P, DT * P], F16, tag="wgt6",
                                           name=f"w6{nm}{p}")
                        nc.sync.dma_start(
                            t[:], srcw[:, p * DT * P : (p + 1) * DT * P])
                        w6[nm].append(t)
                for g in range(NPG):
                    t = wv6_pool.tile([P, DT, 256], F16, tag="wv6",
                                      name=f"wv6r{g}")
                    nc.sync.dma_start(t[:].rearrange("p a b -> p (a b)"),
                                      wv6_in[:, g * 2048 : (g + 1) * 2048])
                    wv6r.append(t)

            def dma_wo():
                if OUTPROJ_FP8:
                    t = wo_pool.tile([P, NPG, 2, D], F8, tag="wo",
                                     name="wo8t")
                    nc.sync.dma_start(t[:].rearrange("p a b c -> p (a b c)"),
                                      wo_in[:])
                    wo8t_box.append(t)
                else:
                    for pr in range(NPAIR):
                        t = wo_pool.tile([P, D], F16, tag="wo",
                                         name=f"wo{pr}")
                        nc.sync.dma_start(t[:],
                                          wo_in[pr * P : (pr + 1) * P, :])
                        wo_tiles.append(t)

            # ---------- units ----------
            def proj8_unit(nm, p, jb):
                """fp8 DR projection of Q/K block jb -> kt/qt [*, jb*SB:]."""
                ps = ps_ms.tile([P, SB], F32, tag="ms", name=f"ps8{nm}")
                for dp in range(NDP):
                    nc.tensor.matmul(
                        ps[:],
                        w8[nm][p][:, dp, :, :],
                        xt[:, jb, dp, :, :],
                        start=(dp == 0),
                        stop=(dp == NDP - 1),
                        perf_mode=DR,
                    )
                dest = qts[p] if nm == "q" else kts[p]
                nc.vector.tensor_scalar_add(
                    dest[:, jb * SB : (jb + 1) * SB], ps[:],
                    biases[nm][:, p : p + 1])

            def v8_unit(g, t):
                """fp8 DR V projection of t-tile t for pair-group g."""
                ps = ps_ms.tile([P, 256], F32, tag="ms", name="psv")
                for dp in range(NDP):
                    nc.tensor.matmul(
                        ps[:],
                        xt[:, t // 4, dp, :, (t % 4) * P : (t % 4 + 1) * P],
                        wv8r[g][:, dp, :, :],
                        start=(dp == 0),
                        stop=(dp == NDP - 1),
                        perf_mode=DR,
                    )
                for u in range(2):
                    dst = v8[2 * g + u][:, t, :].rearrange(
                        "p (h c) -> p h c", h=2)[:, :, 0:DK]
                    nc.vector.tensor_copy(
                        dst,
                        ps[:, u * P : (u + 1) * P].rearrange(
                            "p (h c) -> p h c", c=DK))

            def v6_unit(g, t):
                """fp16 V projection of t-tile t (j0 diag) for group g."""
                ps = ps_ms.tile([P, 256], F32, tag="ms", name="psv6")
                for d in range(DT):
                    nc.tensor.matmul(
                        ps[:],
                        xj0[d][:, t * P : (t + 1) * P],
                        wv6r[g][:, d, :],
                        start=(d == 0),
                        stop=(d == DT - 1),
                    )
                for u in range(2):
                    dst = v6[2 * g + u][:, t, :].rearrange(
                        "p (h c) -> p h c", h=2)[:, :, 0:DK]
                    nc.vector.tensor_copy(
                        dst,
                        ps[:, u * P : (u + 1) * P].rearrange(
                            "p (h c) -> p h c", c=DK))

            def proj6_unit(nm, p):
                """fp16 sidecar projection of s-block 0 -> q6/k6."""
                ps = ps_ms.tile([P, SB], F32, tag="ms", name=f"ps6{nm}")
                for d in range(DT):
                    nc.tensor.matmul(
                        ps[:],
                        w6[nm][p][:, d * P : (d + 1) * P],
                        xj0[d][:],
                        start=(d == 0),
                        stop=(d == DT - 1),
                    )
                dest = q6s[p] if nm == "q" else k6s[p]
                nc.vector.tensor_scalar_add(
                    dest[:], ps[:], biases[nm][:, p : p + 1])

            def outproj_unit(j, m):
                ps = ps_ms.tile([P, SB], F32, tag="ms", name="ps_o")
                if OUTPROJ_FP8:
                    wo8t = wo8t_box[0]
                    for g in range(NPG):
                        nc.tensor.matmul(
                            ps[:],
                            wo8t[:, g, :, m * P : (m + 1) * P],
                            otg[g][:, :, j * SB : (j + 1) * SB],
                            start=(g == 0),
                            stop=(g == NPG - 1),
                            perf_mode=DR,
                        )
                else:
                    for pr in range(NPAIR):
                        nc.tensor.matmul(
                            ps[:],
                            wo_tiles[pr][:, m * P : (m + 1) * P],
                            otg[pr // 2][:, pr % 2, j * SB : (j + 1) * SB],
                            start=(pr == 0),
                            stop=(pr == NPAIR - 1),
                        )
                st = ost_pool.tile([P, SB], F16, tag="ost")
                nc.vector.tensor_copy(st[:], ps[:])
                nc.sync.dma_start(
                    out[m * P : (m + 1) * P, j * SB : (j + 1) * SB], st[:])

            # ---------- schedule machinery ----------
            units = {}
            phase_q = {3: [], 2: [], 1: [], 0: [], -1: []}

            def add_unit(ph, key, fn):
                units[key] = fn
                phase_q[ph].append(key)

            def need(key):
                fn = units.pop(key, None)
                if fn is not None:
                    fn()

            def pop_one(cur):
                for jj in (3, 2, 1, 0, -1):
                    if jj < cur:
                        break
                    q = phase_q[jj]
                    while q:
                        key = q.pop(0)
                        fn = units.pop(key, None)
                        if fn is not None:
                            fn()
                            return True
                return False

            for p in range(NPAIR):
                for jb in range(NSB):
                    add_unit(3, ("k8", p, jb),
                             lambda p=p, jb=jb: proj8_unit("k", p, jb))
                add_unit(3, ("q8", p, 3),
                         lambda p=p: proj8_unit("q", p, 3))
                if p % 2 == 1:
                    g = p // 2
                    for t in range(TT):
                        add_unit(3, ("v8", g, t),
                                 lambda g=g, t=t: v8_unit(g, t))
            for p in range(NPAIR):
                add_unit(3, ("q8", p, 2),
                         lambda p=p: proj8_unit("q", p, 2))
            add_unit(3, ("dma_wo",), dma_wo)
            if not J0_FP8:
                add_unit(2, ("dma6",), dma_sidecar)
            for p in range(NPAIR):
                add_unit(2, ("q8", p, 1),
                         lambda p=p: proj8_unit("q", p, 1))
            if J0_FP8:
                for p in range(NPAIR):
                    add_unit(1, ("q8", p, 0),
                             lambda p=p: proj8_unit("q", p, 0))
            else:
                for p in range(NPAIR):
                    add_unit(2, ("k6", p), lambda p=p: proj6_unit("k", p))
                for p in range(NPAIR):
                    add_unit(1, ("q6", p), lambda p=p: proj6_unit("q", p))
                for g in range(NPG):
                    for t in range(4):
                        add_unit(1, ("v6", g, t),
                                 lambda g=g, t=t: v6_unit(g, t))

            # ---------- attention ----------
            def attention_j(p, j):
                f16 = j == 0 and not J0_FP8
                g = p // 2
                nt = 4 * j + 4
                ktp = k6s[p] if f16 else kts[p]
                qtp = q6s[p] if f16 else qts[p]
                vtp = v6[p] if f16 else v8[p]
                pa = ps_big.tile([P, 2, SB], F32, tag="big", name=f"pa{p}")
                wts = {}
                pend = []

                if f16:
                    need(("k6", p))
                    need(("q6", p))
                else:
                    need(("q8", p, j))

                def av_emit(i):
                    diag = i >= 4 * j
                    r = i - 4 * j
                    w = SB - P * r if diag else SB
                    q = i % 2
                    wt_t = wts[i // 2]
                    last = i == nt - 1
                    if diag:
                        if f16:
                            need(("v6", g, i))
                        else:
                            need(("v8", g, i))
                        c0 = P * r
                        for h in range(2):
                            nc.tensor.matmul(
                                pa[:, h, c0:SB],
                                vtp[:, i, 128 * h : 128 * h + 128],
                                wt_t[:, h, q, 0:w],
                                start=(i == 0),
                                stop=last,
                            )
                    elif q == 1:
                        need(("v8", g, i - 1))
                        need(("v8", g, i))
                        for h in range(2):
                            nc.tensor.matmul(
                                pa[:, h, :],
                                v8[p][:, i - 1 : i + 1,
                                      128 * h : 128 * h + 128],
                                wt_t[:, h, :, :],
                                start=(i == 1),
                                stop=False,
                                perf_mode=DR,
                            )

                for i in range(nt):
                    if not f16:
                        need(("k8", p, i // 4))
                        pop_one(j)
                    elif i % 2 == 1:
                        # j0: only 16 tiles for 8 chunky op fillers -- pop
                        # every other tile so the back half of the phase
                        # (pairs 2-3) still has PE work while exp drains
                        pop_one(j)
                    diag = i >= 4 * j
                    r = i - 4 * j
                    w = SB - P * r if diag else SB
                    qoff = j * SB + (P * r if diag else 0)
                    q = i % 2
                    sc = ps_big.tile([P, 2, SB], F32, tag="big", name="sc")
                    for h in range(2):
                        nc.tensor.matmul(
                            sc[:, h, 0:w],
                            ktp[64 * h : 64 * h + 64, i * P : (i + 1) * P],
                            qtp[64 * h : 64 * h + 64, qoff : qoff + w],
                            start=True, stop=bool(MASK_GPSIMD) or not diag,
                        )
                        if diag and not MASK_GPSIMD:
                            nc.tensor.matmul(
                                sc[:, h, 0:P],
                                cmask[:],
                                ident[:],
                                start=False, stop=True,
                                skip_group_check=True,
                            )
                    if q == 0:
                        if f16:
                            wt_cur = wt6_pool.tile([P, 2, 2, SB], F16,
                                                   tag="wt6", name="wt6c")
                        else:
                            wt_cur = wt_pool.tile([P, 2, 2, SB], F8,
                                                  tag="wt", name="wt8c")
                        wts[i // 2] = wt_cur
                    nc.scalar.activation(
                        wts[i // 2][:, :, q, 0:w], sc[:, :, 0:w], AF.Exp,
                        scale=0.125,
                    )
                    if diag and MASK_GPSIMD:
                        # zero exp(score) where key > query (upper triangle
                        # of the leading 128-col block)
                        nc.gpsimd.affine_select(
                            out=wts[i // 2][:, :, q, 0:P],
                            in_=wts[i // 2][:, :, q, 0:P],
                            compare_op=mybir.AluOpType.is_ge,
                            fill=0.0,
                            base=0,
                            pattern=[[0, 2], [1, P]],
                            channel_multiplier=-1,
                        )
                    pend.append(i)
                    # defer AV by 4 tiles so the PE never waits on exp
                    if len(pend) > 4:
                        av_emit(pend.pop(0))
                for i in pend:
                    av_emit(i)
                den = den_pool.tile([DK, 2, SB], F32, tag="den", name="den")
                nc.vector.tensor_copy(den[:], pa[DK:P, :, :])
                if j > 0:
                    # early pa release: frees the sc/pa slot rotation
                    num = den_pool.tile([DK, 2, SB], F32, tag="num",
                                        name="num")
                    nc.vector.tensor_copy(num[:], pa[0:DK, :, :])
                    numsrc = [num[:, h, :] for h in range(2)]
                else:
                    # tail phase is DVE-bound: skip the extra copy
                    numsrc = [pa[0:DK, h, :] for h in range(2)]
                dst = otg[g]
                rcs = den_pool.tile([DK, 2, SB], F32, tag="den", name="rcs")
                nc.vector.reciprocal_approx_fast(rcs[:], den[:])
                for h in range(2):
                    nc.vector.tensor_tensor(
                        dst[h * DK : (h + 1) * DK, p % 2,
                            j * SB : (j + 1) * SB],
                        numsrc[h],
                        rcs[:, h, :],
                        MUL,
                    )

            # ---------- main loop ----------
            for j in range(NSB - 1, -1, -1):
                for p in range(NPAIR):
                    attention_j(p, j)
                for m in range(DT):
                    add_unit(max(j - 1, -1), ("op", j, m),
                             lambda j=j, m=m: outproj_unit(j, m))
            while pop_one(-1):
                pass

    nc.compile()
    return nc


_NC_CACHE = None


def _get_nc():
    global _NC_CACHE
    if _NC_CACHE is None:
        _NC_CACHE = build_nc()
    return _NC_CACHE


def _core_inputs(x, Wq, bq, Wk, bk, Wv, bv, Wo, c):
    b, g = c // 2, c % 2
    heads = list(range(g * HPC, (g + 1) * HPC))
    out = {}
    xT = np.ascontiguousarray(x[b].T)  # [D, S] f32
    # x8[p, jb, dp, slot, s] = xT[dp*256 + slot*128 + p, jb*512 + s]
    x8 = xT.reshape(NDP, 2, P, NSB, SB).transpose(2, 3, 0, 1, 4)
    out["x8"] = np.ascontiguousarray(
        x8.reshape(P, NSB * NDP * 2 * SB).astype(NPF8))

    def pack_w8(W):
        cols = np.empty((P, NPAIR, NDP, 2, P), dtype=np.float32)
        for p in range(NPAIR):
            hA, hB = heads[2 * p], heads[2 * p + 1]
            Wp = np.concatenate([W[hA], W[hB]], axis=1)  # [1024, 128]
            Wp = Wp.reshape(NDP, 2, P, P)
            cols[:, p] = Wp.transpose(2, 0, 1, 3)
        return np.ascontiguousarray(cols.reshape(P, NPAIR * 1024).astype(NPF8))

    def pack_b(bias):
        cols = np.empty((P, NPAIR), dtype=np.float32)
        for p in range(NPAIR):
            hA, hB = heads[2 * p], heads[2 * p + 1]
            cols[:, p] = np.concatenate([bias[hA], bias[hB]])
        return np.ascontiguousarray(cols)

    out["wq8"] = pack_w8(Wq)
    out["wk8"] = pack_w8(Wk)
    out["bq_n"] = pack_b(bq)
    out["bk_n"] = pack_b(bk)

    # wv8r[p, g, dp, slot, n] = Wv4[g][dp*256 + slot*128 + p, n]
    wv8r = np.empty((P, NPG, NDP, 2, 256), dtype=np.float32)
    wv6r = np.empty((P, NPG, DT, 256), dtype=np.float32)
    for gg in range(NPG):
        hs = heads[4 * gg : 4 * gg + 4]
        Wv4 = np.concatenate([Wv[h] for h in hs], axis=1)  # [1024, 256]
        wv8r[:, gg] = Wv4.reshape(NDP, 2, P, 256).transpose(2, 0, 1, 3)
        if not J0_FP8:
            wv6r[:, gg] = Wv4.reshape(DT, P, 256).transpose(1, 0, 2)
    out["wv8r"] = np.ascontiguousarray(
        wv8r.reshape(P, NPG * NDP * 2 * 256).astype(NPF8))
    if not J0_FP8:
        out["wv6r"] = np.ascontiguousarray(
            wv6r.reshape(P, NPG * DT * 256).astype(np.float16))

    if not J0_FP8:
        out["xj0"] = np.ascontiguousarray(xT[:, 0:SB].astype(np.float16))

    def cat16r(W):
        # [P, pair, d, c]: w6r[p, pr, d, c] = Wpair[d*128 + p, c]
        cols = np.empty((P, NPAIR, DT, P), dtype=np.float32)
        for pr in range(NPAIR):
            Wp = np.concatenate([W[heads[2 * pr]], W[heads[2 * pr + 1]]],
                                axis=1)  # [1024, 128]
            cols[:, pr] = Wp.reshape(DT, P, P).transpose(1, 0, 2)
        return np.ascontiguousarray(
            cols.reshape(P, NPAIR * DT * P).astype(np.float16))
    if not J0_FP8:
        out["wq6r"] = cat16r(Wq)
        out["wk6r"] = cat16r(Wk)

    Wo_core = Wo[:, g * HPC * DK : (g + 1) * HPC * DK].T  # [512, 1024]
    if OUTPROJ_FP8:
        # wo8[p, g, slot, m] = Wo_core[g*256 + slot*128 + p, m]
        wo8 = Wo_core.reshape(NPG, 2, P, D).transpose(2, 0, 1, 3)
        out["wo8"] = np.ascontiguousarray(
            wo8.reshape(P, NPG * 2 * D).astype(NPF8))
    else:
        out["wo_t"] = np.ascontiguousarray(Wo_core.astype(np.float16))
    return out


def kernel(x, Wq, bq, Wk, bk, Wv, bv, Wo, bo, _trace=False, _tmpdir=None):
    x = np.asarray(x, dtype=np.float32)
    nc = _get_nc()
    in_maps = [
        _core_inputs(x, Wq, bq, Wk, bk, Wv, bv, Wo, c) for c in range(NCORES)
    ]
    kw = {}
    if _trace:
        kw = dict(trace=True, tmpdir=_tmpdir)
    res = bass_utils.run_bass_kernel_spmd(
        nc, in_maps, core_ids=list(range(NCORES)), **kw
    )
    # V bias folded here: softmax rows sum to 1 -> +bv passes through AV
    bo_eff = (np.asarray(bo, dtype=np.float32)
              + np.asarray(bv, dtype=np.float32).reshape(-1)
              @ np.asarray(Wo, dtype=np.float32).T)
    out = np.empty((B, S, D), dtype=np.float32)
    for b in range(B):
        part = (res.results[2 * b]["out_part"].astype(np.float32)
                + res.results[2 * b + 1]["out_part"].astype(np.float32))
        out[b] = part.T + bo_eff
    if _trace:
        kernel._last_results = res
    return out
